# revision 16
# baseline (speedup 1.0000x reference)
"""GCN message-passing kernel for Trainium2, 8 NeuronCores (SPMD).

Strategy (graph-parallel, fp8 messages):
- Nodes are protein-contiguous, sharded across 8 cores at protein boundaries
  (16 proteins/core, padded to 6400 nodes/core). Within a core, nodes are
  bin-packed into 50 blocks of 128 balancing incoming-edge counts; slot
  s = blk*128 + pos maps to partition pos, chunk blk everywhere (h, dis,
  table, S, pooling) so aggregation blocks coincide with node chunks.
- Message table is fp8e4m3 at 256B row stride with a 128B payload
  (row = dis[src] * (h @ W)[src]); gathers use elem_size=128/elem_step=256
  (bass's %256 payload assert is bypassed via direct InstDMAGatherAnt
  construction - verified byte-exact on hardware), halving per-edge DMA
  cost vs bf16.
- Aggregation is node-major: acc[dst,feat] = S^T @ msgs with S a 0/1
  one-hot (exact in fp8) via DoubleRow fp8 matmuls (256 slots each);
  dis[dst] is applied EXACTLY by the relu epilogue's per-partition ACT
  scale; conv bias enters through one bias slot per block whose S column
  is 1/dis[dst] and whose table row is conv_b (rewritten per layer at the
  reserved slot (pos 0, blk 49) of every core).
- h is kept bf16 both node-major (epilogue output; feeds readout) and
  feature-major (one PE transpose per chunk per layer; feeds the bf16
  h @ W matmuls computed directly node-major as lhsT=h_fm, rhs=W).
- Readout: scores via DVE mul+reduce against a broadcast att_w row,
  global-shift masked softmax, fused pooling matmuls with
  rhs = [h | ex*h | ex] per chunk, final projection per core.
"""
import os
import numpy as np
import ml_dtypes

DBG_LAYERS = int(os.environ.get("GCN_DBG_LAYERS", "4"))
DBG_NO_COLL = os.environ.get("GCN_DBG_NO_COLL", "") == "1"
DBG_NO_GATHER = os.environ.get("GCN_DBG_NO_GATHER", "") == "1"
DBG_NO_READOUT = os.environ.get("GCN_DBG_NO_READOUT", "") == "1"
DBG_DUMP_H = os.environ.get("GCN_DBG_DUMP_H", "") == "1"
DBG_DUMP_TB = os.environ.get("GCN_DBG_DUMP_TB", "") == "1"

import concourse.bacc as bacc
import concourse.tile as tile
import concourse.tile_utils as tile_utils
from concourse import mybir
from concourse.bass_utils import run_bass_kernel_spmd
from concourse.masks import make_identity

bf16 = ml_dtypes.bfloat16
E4 = ml_dtypes.float8_e4m3
AF = mybir.ActivationFunctionType

NC = 8
D = 128
L = 4
B = 128
PPC = B // NC          # proteins per core
NPAD = 6400            # padded nodes per core
NPADG = NC * NPAD      # global padded rows
NTB = NPAD // 128      # 50 chunks of 128 nodes == aggregation blocks
NBLK = NTB
TW = 128               # dst nodes per aggregation block
LO_BOUND = 32000       # lo gather covers rows [0, 32000)
HI_BASE = 18560        # hi gather covers rows [18560, 51200): 32639 <= int16
GCH = 8192             # gather slots per dma_gather instruction (64 cols)
BIAS_SLOT = NBLK - 1   # reserved slot (pos 0, blk 49) on every core

f32 = mybir.dt.float32
bft = mybir.dt.bfloat16
fp8 = mybir.dt.float8e4
i16 = mybir.dt.int16


# ---------------------------------------------------------------- host prep

def _pack_idx(vals, slots):
    """int16 gather index layout: position i -> partition i%16, col i//16,
    replicated across the 128 partitions."""
    assert len(vals) == slots and slots % 16 == 0
    arr = np.asarray(vals, np.int16).reshape(slots // 16, 16).T  # [16, s//16]
    return np.ascontiguousarray(np.tile(arr, (8, 1)))


def _even_ceil(x):
    t = int(np.ceil(x / 128))
    return t + (t % 2)


def _host_prep(x, edge_index, batch, lysine_mask):
    N = x.shape[0]
    src = np.asarray(edge_index[0], np.int64)
    dst = np.asarray(edge_index[1], np.int64)
    batch = np.asarray(batch, np.int64)

    pcounts = np.bincount(batch, minlength=B)
    pstart = np.concatenate([[0], np.cumsum(pcounts)])
    cstart = pstart[np.arange(NC) * PPC]
    cend = pstart[(np.arange(NC) + 1) * PPC]
    ncore = cend - cstart
    assert ncore.max() <= NPAD - 1, f"core node count {ncore.max()} > {NPAD-1}"
    assert pcounts.max() <= 128 * NTB

    deg = np.bincount(dst, minlength=N).astype(np.float64) + 1.0
    dis = (1.0 / np.sqrt(deg)).astype(np.float32)
    core_of = np.searchsorted(cend, np.arange(N), side="right")

    # --- per-core node packing into NBLK blocks of 128, balancing in-slot
    # (in-edges + self) counts per block; (pos 127, blk 49) is reserved.
    blk = np.zeros(N, np.int64)
    pos = np.zeros(N, np.int64)
    for c in range(NC):
        nodes = np.arange(cstart[c], cend[c])
        tot = deg[nodes]
        order = np.argsort(-tot, kind="stable")
        caps = np.full(NBLK, 128, np.int64)
        caps[NBLK - 1] = 127
        loads = np.zeros(NBLK)
        cnts = np.zeros(NBLK, np.int64)
        for i in order:
            masked = np.where(cnts < caps, loads, np.inf)
            b = int(np.argmin(masked))
            blk[nodes[i]] = b
            # (pos 0, blk 49) is the reserved bias slot on every core
            pos[nodes[i]] = cnts[b] + (1 if b == NBLK - 1 else 0)
            cnts[b] += 1
            loads[b] += tot[i]
    slot = blk * 128 + pos                    # local pi slot
    grow = core_of * NPAD + pos * NTB + blk   # global table row

    # --- edge list: real edges + self edges + one bias pseudo-edge per
    # (core, block) (dst col -1). Bias row is the reserved slot's row.
    e_src_row = np.concatenate([grow[src], grow])
    e_dst = np.concatenate([dst, np.arange(N)])
    e_core = core_of[e_dst]
    e_blk = blk[e_dst]
    e_col = pos[e_dst]
    bias_core = np.repeat(np.arange(NC), NBLK)
    bias_blk = np.tile(np.arange(NBLK), NC)
    bias_row_of_core = np.arange(NC) * NPAD + (NBLK - 1)  # (pos 0, blk 49)
    e_src_row = np.concatenate([e_src_row, bias_row_of_core[bias_core]])
    e_core = np.concatenate([e_core, bias_core])
    e_blk = np.concatenate([e_blk, bias_blk])
    e_col = np.concatenate([e_col, np.full(NC * NBLK, -1, np.int64)])

    cls = np.where(e_src_row < HI_BASE, 0,
                   np.where(e_src_row < LO_BOUND, 1, 2))
    key = e_core * NBLK + e_blk
    nl0 = np.bincount(key[cls == 0], minlength=NC * NBLK)
    nf = np.bincount(key[cls == 1], minlength=NC * NBLK)
    tot_cb = np.bincount(key, minlength=NC * NBLK)

    best = None
    for LO_T in range(_even_ceil(nl0.max()), _even_ceil(nl0.max()) + 8, 2):
        lo_fill = np.minimum(LO_T * 128, nl0 + nf)
        HI_T = _even_ceil((tot_cb - lo_fill).max())
        if best is None or LO_T + HI_T < best[0] + best[1]:
            best = (LO_T, HI_T)
    LO_T, HI_T = best
    NT = LO_T + HI_T

    per_core = []
    for c in range(NC):
        m = e_core == c
        rows_e, blk_e, col_e, cls_e = (
            e_src_row[m], e_blk[m], e_col[m], cls[m])
        order = np.lexsort((col_e, cls_e, blk_e))
        rows_e, blk_e, col_e, cls_e = (
            rows_e[order], blk_e[order], col_e[order], cls_e[order])
        bstart = np.searchsorted(blk_e, np.arange(NBLK))
        bend = np.searchsorted(blk_e, np.arange(NBLK), side="right")

        nodes = np.arange(cstart[c], cend[c])
        # dis / inv-dis in pi layout (pads -> 1 / 0)
        dis_nm = np.ones((128, NTB), np.float32)
        inv_nm = np.zeros((128, NTB), np.float32)
        dis_nm[pos[nodes], blk[nodes]] = dis[nodes]
        inv_nm[pos[nodes], blk[nodes]] = 1.0 / dis[nodes]

        lo_idx = np.zeros(NBLK * LO_T * 128, np.int64)
        hi_idx = np.zeros(NBLK * HI_T * 128, np.int64)  # already HI_BASE-offset
        s_all = np.zeros((128, NBLK * NT * 128), np.float32)
        for b in range(NBLK):
            sl = slice(bstart[b], bend[b])
            r_b, c_b, k_b = rows_e[sl], col_e[sl], cls_e[sl]
            n = len(r_b)
            n0 = int((k_b == 0).sum())
            nfb = int((k_b == 1).sum())
            take = min(LO_T * 128 - n0, nfb)
            assert take >= 0, f"block lo overflow {n0} > {LO_T*128}"
            nlo = n0 + take
            nhi = n - nlo
            assert nhi <= HI_T * 128
            for stream, cnt, off, idxarr, base_t, ibase in (
                (0, nlo, 0, lo_idx, 0, 0),
                (1, nhi, nlo, hi_idx, LO_T, HI_BASE),
            ):
                if cnt == 0:
                    continue
                rr = r_b[off:off + cnt] - ibase
                cc = c_b[off:off + cnt]
                T = LO_T if stream == 0 else HI_T
                idxarr[b * T * 128: b * T * 128 + cnt] = rr
                k = np.arange(cnt)
                p = k % 128
                t = base_t + k // 128
                scol = (b * NT + t) * 128
                real = cc >= 0
                s_all[p[real], scol[real] + cc[real]] = 1.0
                for j in np.flatnonzero(~real):  # bias slots (dense column)
                    # table bias row holds 16*conv_b (fp8 normal range);
                    # S carries the 1/16 to keep values out of subnormals
                    s_all[p[j], scol[j]:scol[j] + 128] = inv_nm[:, b] / 16.0

        x_t = np.zeros((D, NPAD), np.float32)
        x_t[:, slot[nodes]] = np.asarray(x[nodes], np.float32).T

        lens = pcounts[c * PPC:(c + 1) * PPC]
        starts = np.concatenate([[0], np.cumsum(lens)])[:-1]
        q = np.arange(ncore[c])
        pj = np.searchsorted(starts, q, side="right") - 1
        pone = np.zeros((128, NTB * PPC), bf16)
        pone[pos[nodes], blk[nodes] * PPC + pj] = 1.0
        lys_nm = np.zeros((128, NTB), np.float32)
        lys_nm[pos[nodes], blk[nodes]] = np.asarray(
            lysine_mask[nodes], np.float32)

        per_core.append(dict(
            x_t=x_t.astype(bf16),
            s_all=s_all.astype(E4),
            idx_lo=_pack_idx(lo_idx, NBLK * LO_T * 128),
            idx_hi=_pack_idx(hi_idx, NBLK * HI_T * 128),
            dis_nm=dis_nm,
            pone=pone,
            cnt_col=lens.astype(np.float32).reshape(PPC, 1),
            lys_nm=lys_nm,
        ))
    return per_core, LO_T, HI_T, NT


# ---------------------------------------------------------------- program

def _dma_gather_128(nc, out_ap, in_ap, idxs_ap, num_idxs):
    """dma_gather with a 128B payload on a 256B-stride table (elem_size=128
    fp8, elem_step=256). Bypasses bass's %256 payload assert; verified
    byte-exact on hardware."""
    g = nc.gpsimd
    _in_ap = g.lower_ap_dma(in_ap, for_custom_bir_dma=True)
    _idxs_ap = g.lower_ap(idxs_ap)
    _out_ap = g.lower_ap(out_ap)
    return g.add_instruction(mybir.InstDMAGatherAnt(
        name=g.bass.get_next_instruction_name(),
        ins=[*_in_ap, _idxs_ap, g.lower_val_access(g.to_reg(num_idxs))],
        outs=[_out_ap],
        transpose=False, num_idxs=num_idxs, elem_size=128,
        stride_bytes_256=1, gen_mode=0, single_packet=False,
        queue_num=0, sbuf_tokens_per_rank=0, sbuf_free_dim_per_rank=0,
        sbuf_free_dim_pad_per_rank=0, sbuf_byte_offset=0))


def _build_program(LO_T, HI_T, NT):
    tile_utils.max_sbuf_usage = 204 * 1024
    nc = bacc.Bacc("TRN2", target_bir_lowering=False, num_devices=NC,
                   num_swdge_queues=2)

    din = {}
    for name, shape, dt in [
        ("x_t", [D, NPAD], bft),
        ("s_all", [128, NBLK * NT * 128], fp8),
        ("idx_lo", [128, NBLK * LO_T * 8], i16),
        ("idx_hi", [128, NBLK * HI_T * 8], i16),
        ("dis_nm", [128, NTB], f32),
        ("pone", [128, NTB * PPC], bft),
        ("cnt_col", [PPC, 1], f32),
        ("lys_nm", [128, NTB], f32),
        ("convw", [D, L * D], bft),
        ("convb_pre", [128, L * D], fp8),
        ("attw_row", [1, D], f32),
        ("outw", [D, 64], f32),
        ("outb", [64, 1], f32),
    ]:
        din[name] = nc.dram_tensor(name, shape, dt, kind="ExternalInput")
    out_t = nc.dram_tensor("out_t", [64, PPC], f32, kind="ExternalOutput")
    out_h = None
    if DBG_DUMP_H:
        out_h = nc.dram_tensor("out_h", [128, NPAD], bft,
                               kind="ExternalOutput")
    out_tb = None
    if DBG_DUMP_TB:
        out_tb = nc.dram_tensor("out_tb", [128, NTB * 128], fp8,
                                kind="ExternalOutput")

    LO_SLOTS = NBLK * LO_T * 128
    HI_SLOTS = NBLK * HI_T * 128

    with tile.TileContext(nc) as tc:
        with (
            tc.tile_pool(name="glob", bufs=1) as gp,
            tc.tile_pool(name="dram", bufs=1, space="DRAM") as dram,
        ):
            # resident SBUF state
            h_fm = gp.tile([D, NPAD], bft, name="h_fm")
            nc.sync.dma_start(h_fm[:], din["x_t"][:])
            h_nm = gp.tile([128, NTB, 128], bft, name="h_nm")
            staging = gp.tile([128, NTB, 128], fp8, name="staging")
            s_sb = gp.tile([128, NBLK * NT * 128], fp8, name="s_sb")
            nc.sync.dma_start(s_sb[:], din["s_all"][:])
            dis_nm = gp.tile([128, NTB], f32)
            nc.sync.dma_start(dis_nm[:], din["dis_nm"][:])
            idx_lo = gp.tile([128, LO_SLOTS // 16], i16)
            nc.sync.dma_start(idx_lo[:], din["idx_lo"][:])
            idx_hi = gp.tile([128, HI_SLOTS // 16], i16)
            nc.sync.dma_start(idx_hi[:], din["idx_hi"][:])
            convw = gp.tile([D, L * D], bft)
            nc.sync.dma_start(convw[:], din["convw"][:])
            convb_pre = gp.tile([128, L * D], fp8)
            nc.sync.dma_start(convb_pre[:], din["convb_pre"][:])

            stripe = dram.tile([NPAD, 256], fp8)
            hws_full = dram.tile([NPADG, 256], fp8)
            tident = gp.tile([128, 128], bft)
            make_identity(nc, tident[:])

            # ---------------- GCN layers
            with (
                tc.tile_pool(name="msgs", bufs=2) as mp,
                tc.tile_pool(name="ps_w", bufs=2, space="PSUM") as ps_w,
                tc.tile_pool(name="ps_agg", bufs=4, space="PSUM") as ps_agg,
                tc.tile_pool(name="ps_tr", bufs=2, space="PSUM") as ps_tr,
            ):
                for layer in range(DBG_LAYERS):
                    # table: hws_nm = dis[src] * (h @ W), node-major fp8.
                    # pw_nm = h_fm_chunk^T @ W directly node-major.
                    for b in range(NTB):
                        pw = ps_w.tile([128, D], f32, tag="wmm")
                        nc.tensor.matmul(
                            out=pw[:],
                            lhsT=h_fm[:, b * 128:(b + 1) * 128],
                            rhs=convw[:, layer * D:(layer + 1) * D],
                            start=True, stop=True)
                        nc.scalar.activation(
                            staging[:, b, :], pw[:], AF.Copy,
                            scale=dis_nm[:, b:b + 1])
                    # bias table row at the reserved slot (pos 0, blk 49)
                    nc.vector.tensor_copy(
                        staging[0:1, NBLK - 1, :],
                        convb_pre[0:1, layer * D:(layer + 1) * D])
                    spm = stripe[:, 0:128].rearrange("(p k) f -> p k f", k=NTB)
                    nc.sync.dma_start(spm, staging[:])
                    if DBG_NO_COLL:
                        nc.gpsimd.dma_start(hws_full[0:NPAD, :], stripe[:])
                    else:
                        nc.gpsimd.collective_compute(
                            "AllGather", mybir.AluOpType.bypass,
                            replica_groups=[list(range(NC))],
                            ins=[stripe.opt()], outs=[hws_full.opt()])

                    # gathers issued lazily in consumption order; aggregate
                    # via DoubleRow fp8 matmuls; relu epilogue with exact
                    # dis[dst] as the ACT per-partition scale.
                    lo_chunks, hi_chunks = {}, {}

                    def get_chunk(done, ci, slots, idx, base_hi, tg):
                        if ci not in done:
                            s0 = ci * GCH
                            n = min(GCH, slots - s0)
                            m = mp.tile([128, GCH // 128, 128], fp8, tag=tg)
                            if DBG_NO_GATHER:
                                nc.vector.memset(m[:], 0.0)
                            else:
                                src_ap = (hws_full[HI_BASE:, 0:128] if base_hi
                                          else hws_full[:, 0:128])
                                _dma_gather_128(
                                    nc, m[:, : n // 128, :], src_ap,
                                    idx[:, s0 // 16:(s0 + n) // 16], n)
                            done[ci] = m
                        return done[ci]

                    for b in range(NBLK):
                        acc = ps_agg.tile([128, D], f32, tag="agg")
                        for j in range(NT // 2):
                            if 2 * j < LO_T:
                                col = b * LO_T + 2 * j
                                mm = get_chunk(
                                    lo_chunks, col // (GCH // 128), LO_SLOTS,
                                    idx_lo, False, "mlo")
                            else:
                                col = b * HI_T + 2 * (j - LO_T // 2)
                                mm = get_chunk(
                                    hi_chunks, col // (GCH // 128), HI_SLOTS,
                                    idx_hi, True, "mhi")
                            cc = col % (GCH // 128)
                            sc0 = (b * NT + 2 * j) * 128
                            nc.tensor.matmul(
                                out=acc[:],
                                lhsT=s_sb[:, sc0:sc0 + 256].rearrange(
                                    "p (i d) -> p i d", i=2),
                                rhs=mm[:, cc:cc + 2, :],
                                start=(j == 0), stop=(j == NT // 2 - 1),
                                perf_mode=mybir.MatmulPerfMode.DoubleRow)
                        nc.scalar.activation(
                            h_nm[:, b, :], acc[:], AF.Relu,
                            scale=dis_nm[:, b:b + 1])
                        if layer < DBG_LAYERS - 1 or DBG_DUMP_H:
                            pt = ps_tr.tile([128, 128], bft, tag="ptr")
                            nc.tensor.transpose(
                                out=pt[:], in_=h_nm[:, b, :],
                                identity=tident[:])
                            nc.vector.tensor_copy(
                                h_fm[:, b * 128:(b + 1) * 128], pt[:])

            if DBG_DUMP_H:
                for b in range(NTB):
                    nc.gpsimd.dma_start(
                        out_h[:, b * 128:(b + 1) * 128],
                        h_fm[:, b * 128:(b + 1) * 128])
            if DBG_DUMP_TB:
                nc.gpsimd.dma_start(
                    out_tb[:].rearrange("p (k f) -> p k f", k=NTB),
                    staging[:])

            if DBG_NO_READOUT:
                with tc.tile_pool(name="r0", bufs=1) as r0:
                    oz = r0.tile([64, PPC], f32)
                    nc.vector.tensor_copy(oz[:], h_nm[0:64, 0, 0:PPC])
                    nc.gpsimd.dma_start(out_t[:], oz[:])

            if not DBG_NO_READOUT:
                with (
                    tc.tile_pool(name="r_sb", bufs=1) as rp,
                    tc.tile_pool(name="r2", bufs=2) as rp2,
                    tc.tile_pool(name="ps_r", bufs=2, space="PSUM") as ps_r,
                    tc.tile_pool(name="ps_p", bufs=1, space="PSUM") as ps_p,
                ):
                    ident = rp.tile([128, 128], f32)
                    make_identity(nc, ident[:])
                    ones_r = rp.tile([1, 128], f32)
                    nc.vector.memset(ones_r[:], 1.0)
                    attw = rp.tile([1, D], f32)
                    nc.sync.dma_start(attw[:], din["attw_row"][:])

                    # att_w broadcast to all partitions (ones outer product)
                    psat = ps_r.tile([128, D], f32, tag="tr")
                    nc.tensor.matmul(out=psat[:], lhsT=ones_r[:],
                                     rhs=attw[:], start=True, stop=True)
                    attrep = rp.tile([128, D], bft)
                    nc.vector.tensor_copy(attrep[:], psat[:])

                    # scores node-major via DVE mul + row-reduce
                    sc_nm = rp.tile([128, NTB], f32)
                    for t in range(NTB):
                        tmp = rp2.tile([128, D], bft, tag="sc")
                        nc.vector.tensor_mul(tmp[:], h_nm[:, t, :], attrep[:])
                        nc.vector.tensor_reduce(
                            out=sc_nm[:, t:t + 1], in_=tmp[:],
                            axis=mybir.AxisListType.X, op=mybir.AluOpType.add)

                    # global-shift masked softmax pieces (shift-invariant)
                    colmax = rp.tile([128, 1], f32)
                    nc.vector.tensor_reduce(
                        out=colmax[:], in_=sc_nm[:],
                        axis=mybir.AxisListType.X, op=mybir.AluOpType.max)
                    ptm = ps_r.tile([128, 128], f32, tag="tr")
                    nc.tensor.transpose(
                        out=ptm[0:1, :], in_=colmax[:], identity=ident[:])
                    rowmax = rp.tile([1, 128], f32)
                    nc.vector.tensor_copy(rowmax[:], ptm[0:1, :])
                    gmax = rp.tile([1, 1], f32)
                    nc.vector.tensor_reduce(
                        out=gmax[:], in_=rowmax[:],
                        axis=mybir.AxisListType.X, op=mybir.AluOpType.max)
                    ngmax = rp.tile([1, 1], f32)
                    nc.vector.tensor_scalar_mul(ngmax[:], gmax[:], -1.0)
                    psng = ps_p.tile([128, 1], f32, tag="ng")
                    nc.tensor.matmul(out=psng[:], lhsT=ones_r[:],
                                     rhs=ngmax[:], start=True, stop=True)
                    ngcol = rp.tile([128, 1], f32)
                    nc.vector.tensor_copy(ngcol[:], psng[:])
                    exm = rp.tile([128, NTB], f32)
                    nc.scalar.activation(exm[:], sc_nm[:], AF.Exp,
                                         bias=ngcol[:])
                    lys_nm = rp.tile([128, NTB], f32)
                    nc.sync.dma_start(lys_nm[:], din["lys_nm"][:])
                    nc.vector.tensor_mul(exm[:], exm[:], lys_nm[:])

                    # fused pooling matmuls: rhs = [h | ex*h | ex]
                    pone = rp.tile([128, NTB * PPC], bft)
                    nc.sync.dma_start(pone[:], din["pone"][:])
                    pall = ps_p.tile([PPC, 257], f32, tag="pall")
                    for t in range(NTB):
                        rh = rp2.tile([128, 257], bft, tag="rh")
                        nc.vector.tensor_copy(rh[:, 0:128], h_nm[:, t, :])
                        nc.vector.tensor_scalar_mul(
                            rh[:, 128:256], h_nm[:, t, :], exm[:, t:t + 1])
                        nc.vector.tensor_copy(
                            rh[:, 256:257], exm[:, t:t + 1])
                        nc.tensor.matmul(
                            out=pall[:],
                            lhsT=pone[:, t * PPC:(t + 1) * PPC], rhs=rh[:],
                            start=(t == 0), stop=(t == NTB - 1))

                    # c_j = 1/(max(cnt,1)*sqrt(cnt+1e-6)); rden = 1/max(dn,eps)
                    cnt = rp.tile([PPC, 1], f32)
                    nc.sync.dma_start(cnt[:], din["cnt_col"][:])
                    cg = rp.tile([PPC, 1], f32)
                    nc.vector.tensor_scalar_max(cg[:], cnt[:], 1.0)
                    cnte = rp.tile([PPC, 1], f32)
                    nc.vector.tensor_scalar_add(cnte[:], cnt[:], 1.0e-6)
                    sq = rp.tile([PPC, 1], f32)
                    nc.scalar.activation(sq[:], cnte[:], AF.Sqrt)
                    mm_ = rp.tile([PPC, 1], f32)
                    nc.vector.tensor_mul(mm_[:], cg[:], sq[:])
                    cj = rp.tile([PPC, 1], f32)
                    nc.vector.reciprocal(cj[:], mm_[:])
                    dg = rp.tile([PPC, 1], f32)
                    nc.vector.tensor_scalar_max(
                        dg[:], pall[:, 256:257], 1.0e-30)
                    rden = rp.tile([PPC, 1], f32)
                    nc.vector.reciprocal(rden[:], dg[:])

                    pre = rp.tile([PPC, 128], f32)
                    nc.vector.tensor_scalar_mul(pre[:], pall[:, 0:128], cj[:])
                    lw = rp.tile([PPC, 128], f32)
                    nc.vector.tensor_scalar_mul(
                        lw[:], pall[:, 128:256], rden[:])
                    nc.vector.tensor_add(pre[:], pre[:], lw[:])

                    # out^T = outw^T @ pre^T + outb
                    ptp = ps_r.tile([128, 128], f32, tag="tr")
                    nc.tensor.transpose(
                        out=ptp[:, 0:PPC], in_=pre[:],
                        identity=ident[0:PPC, 0:PPC])
                    preT = rp.tile([128, PPC], f32)
                    nc.vector.tensor_copy(preT[:], ptp[:, 0:PPC])
                    outw = rp.tile([D, 64], f32)
                    nc.sync.dma_start(outw[:], din["outw"][:])
                    outb = rp.tile([64, 1], f32)
                    nc.sync.dma_start(outb[:], din["outb"][:])
                    pso = ps_p.tile([64, PPC], f32, tag="o")
                    nc.tensor.matmul(
                        out=pso[:], lhsT=outw[:], rhs=preT[:],
                        start=True, stop=True)
                    osb = rp.tile([64, PPC], f32)
                    nc.vector.tensor_scalar_add(osb[:], pso[:], outb[:])
                    nc.gpsimd.dma_start(out_t[:], osb[:])

    nc.compile()
    return nc


# ---------------------------------------------------------------- entry

def kernel(**inputs):
    x = np.asarray(inputs["x"], np.float32)
    edge_index = np.asarray(inputs["edge_index"])
    batch = np.asarray(inputs["batch"])
    lysine_mask = np.asarray(inputs["lysine_mask"])
    conv_w = np.asarray(inputs["conv_w"], np.float32)
    conv_b = np.asarray(inputs["conv_b"], np.float32)
    att_w = np.asarray(inputs["att_w"], np.float32)
    out_w = np.asarray(inputs["out_w"], np.float32)
    out_b = np.asarray(inputs["out_b"], np.float32)

    per_core, LO_T, HI_T, NT = _host_prep(x, edge_index, batch, lysine_mask)

    convw = np.ascontiguousarray(
        np.concatenate([conv_w[i] for i in range(L)], axis=1)).astype(bf16)
    convb_pre = np.tile(
        np.concatenate([16.0 * conv_b[i] for i in range(L)]).astype(E4),
        (128, 1))
    shared = dict(
        convw=convw, convb_pre=convb_pre,
        attw_row=att_w.reshape(1, D).astype(np.float32),
        outw=out_w.astype(np.float32),
        outb=out_b.reshape(64, 1).astype(np.float32),
    )
    in_maps = []
    for c in range(NC):
        pc = per_core[c]
        in_maps.append({
            "x_t": pc["x_t"], "s_all": pc["s_all"],
            "idx_lo": pc["idx_lo"], "idx_hi": pc["idx_hi"],
            "dis_nm": pc["dis_nm"],
            "pone": pc["pone"], "cnt_col": pc["cnt_col"],
            "lys_nm": pc["lys_nm"], **shared,
        })

    nc_prog = _build_program(LO_T, HI_T, NT)
    trace = os.environ.get("GCN_TRACE", "") == "1"
    res = run_bass_kernel_spmd(
        nc_prog, in_maps, core_ids=list(range(NC)), trace=trace)
    if trace:
        import kernel as _self
        _self.LAST_RESULT = res
        print("HW exec time:", res.exec_time_ns, "ns")
    out = np.concatenate(
        [np.asarray(res.results[c]["out_t"], np.float32).T for c in range(NC)],
        axis=0)
    return out


# revision 18
# speedup vs baseline: 1.1790x; 1.1790x over previous
"""GCN message-passing kernel for Trainium2, 8 NeuronCores (SPMD).

Strategy (graph-parallel, fp8 messages):
- Nodes are protein-contiguous, sharded across 8 cores at protein boundaries
  (16 proteins/core, padded to 6400 nodes/core). Within a core, nodes are
  bin-packed into 50 blocks of 128 balancing incoming-edge counts; slot
  s = blk*128 + pos maps to partition pos, chunk blk everywhere (h, dis,
  table, S, pooling) so aggregation blocks coincide with node chunks.
- Message table is fp8e4m3 at 256B row stride with a 128B payload
  (row = dis[src] * (h @ W)[src]); gathers use elem_size=128/elem_step=256
  (bass's %256 payload assert is bypassed via direct InstDMAGatherAnt
  construction - verified byte-exact on hardware), halving per-edge DMA
  cost vs bf16.
- Aggregation is node-major: acc[dst,feat] = S^T @ msgs with S a 0/1
  one-hot (exact in fp8) via DoubleRow fp8 matmuls (256 slots each);
  dis[dst] is applied EXACTLY by the relu epilogue's per-partition ACT
  scale; conv bias enters through one bias slot per block whose S column
  is 1/dis[dst] and whose table row is conv_b (rewritten per layer at the
  reserved slot (pos 0, blk 49) of every core).
- h is kept bf16 both node-major (epilogue output; feeds readout) and
  feature-major (one PE transpose per chunk per layer; feeds the bf16
  h @ W matmuls computed directly node-major as lhsT=h_fm, rhs=W).
- Readout: scores via DVE mul+reduce against a broadcast att_w row,
  global-shift masked softmax, fused pooling matmuls with
  rhs = [h | ex*h | ex] per chunk, final projection per core.
"""
import os
import numpy as np
import ml_dtypes

DBG_LAYERS = int(os.environ.get("GCN_DBG_LAYERS", "4"))
DBG_NO_COLL = os.environ.get("GCN_DBG_NO_COLL", "") == "1"
DBG_NO_GATHER = os.environ.get("GCN_DBG_NO_GATHER", "") == "1"
DBG_NO_READOUT = os.environ.get("GCN_DBG_NO_READOUT", "") == "1"
DBG_DUMP_H = os.environ.get("GCN_DBG_DUMP_H", "") == "1"
DBG_DUMP_TB = os.environ.get("GCN_DBG_DUMP_TB", "") == "1"

import concourse.bacc as bacc
import concourse.tile as tile
import concourse.tile_utils as tile_utils
from concourse import mybir
from concourse.bass_utils import run_bass_kernel_spmd
from concourse.masks import make_identity

bf16 = ml_dtypes.bfloat16
E4 = ml_dtypes.float8_e4m3
AF = mybir.ActivationFunctionType

NC = 8
D = 128
L = 4
B = 128
PPC = B // NC          # proteins per core
NPAD = 6400            # padded nodes per core
NPADG = NC * NPAD      # global padded rows
NTB = NPAD // 128      # 50 chunks of 128 nodes == aggregation blocks
NBLK = NTB
TW = 128               # dst nodes per aggregation block
LO_BOUND = 32000       # lo gather covers rows [0, 32000)
HI_BASE = 18560        # hi gather covers rows [18560, 51200): 32639 <= int16
GCH = 8192             # gather slots per dma_gather instruction (64 cols)
BIAS_SLOT = NBLK - 1   # reserved slot (pos 0, blk 49) on every core

f32 = mybir.dt.float32
bft = mybir.dt.bfloat16
fp8 = mybir.dt.float8e4
i16 = mybir.dt.int16


# ---------------------------------------------------------------- host prep

def _pack_idx(vals, slots):
    """int16 gather index layout: position i -> partition i%16, col i//16,
    replicated across the 128 partitions."""
    assert len(vals) == slots and slots % 16 == 0
    arr = np.asarray(vals, np.int16).reshape(slots // 16, 16).T  # [16, s//16]
    return np.ascontiguousarray(np.tile(arr, (8, 1)))


def _even_ceil(x):
    t = int(np.ceil(x / 128))
    return t + (t % 2)


def _host_prep(x, edge_index, batch, lysine_mask):
    N = x.shape[0]
    src = np.asarray(edge_index[0], np.int64)
    dst = np.asarray(edge_index[1], np.int64)
    batch = np.asarray(batch, np.int64)

    pcounts = np.bincount(batch, minlength=B)
    pstart = np.concatenate([[0], np.cumsum(pcounts)])
    cstart = pstart[np.arange(NC) * PPC]
    cend = pstart[(np.arange(NC) + 1) * PPC]
    ncore = cend - cstart
    assert ncore.max() <= NPAD - 1, f"core node count {ncore.max()} > {NPAD-1}"
    assert pcounts.max() <= 128 * NTB

    deg = np.bincount(dst, minlength=N).astype(np.float64) + 1.0
    dis = (1.0 / np.sqrt(deg)).astype(np.float32)
    core_of = np.searchsorted(cend, np.arange(N), side="right")

    # --- per-core node packing into NBLK blocks of 128, balancing in-slot
    # (in-edges + self) counts per block; (pos 127, blk 49) is reserved.
    blk = np.zeros(N, np.int64)
    pos = np.zeros(N, np.int64)
    for c in range(NC):
        nodes = np.arange(cstart[c], cend[c])
        tot = deg[nodes]
        order = np.argsort(-tot, kind="stable")
        caps = np.full(NBLK, 128, np.int64)
        caps[NBLK - 1] = 127
        loads = np.zeros(NBLK)
        cnts = np.zeros(NBLK, np.int64)
        for i in order:
            masked = np.where(cnts < caps, loads, np.inf)
            b = int(np.argmin(masked))
            blk[nodes[i]] = b
            # (pos 0, blk 49) is the reserved bias slot on every core
            pos[nodes[i]] = cnts[b] + (1 if b == NBLK - 1 else 0)
            cnts[b] += 1
            loads[b] += tot[i]
    slot = blk * 128 + pos                    # local pi slot
    grow = core_of * NPAD + pos * NTB + blk   # global table row

    # --- edge list: real edges + self edges + one bias pseudo-edge per
    # (core, block) (dst col -1). Bias row is the reserved slot's row.
    e_src_row = np.concatenate([grow[src], grow])
    e_dst = np.concatenate([dst, np.arange(N)])
    e_core = core_of[e_dst]
    e_blk = blk[e_dst]
    e_col = pos[e_dst]
    bias_core = np.repeat(np.arange(NC), NBLK)
    bias_blk = np.tile(np.arange(NBLK), NC)
    bias_row_of_core = np.arange(NC) * NPAD + (NBLK - 1)  # (pos 0, blk 49)
    e_src_row = np.concatenate([e_src_row, bias_row_of_core[bias_core]])
    e_core = np.concatenate([e_core, bias_core])
    e_blk = np.concatenate([e_blk, bias_blk])
    e_col = np.concatenate([e_col, np.full(NC * NBLK, -1, np.int64)])

    cls = np.where(e_src_row < HI_BASE, 0,
                   np.where(e_src_row < LO_BOUND, 1, 2))
    key = e_core * NBLK + e_blk
    nl0 = np.bincount(key[cls == 0], minlength=NC * NBLK)
    nf = np.bincount(key[cls == 1], minlength=NC * NBLK)
    tot_cb = np.bincount(key, minlength=NC * NBLK)

    best = None
    for LO_T in range(_even_ceil(nl0.max()), _even_ceil(nl0.max()) + 8, 2):
        lo_fill = np.minimum(LO_T * 128, nl0 + nf)
        HI_T = _even_ceil((tot_cb - lo_fill).max())
        if best is None or LO_T + HI_T < best[0] + best[1]:
            best = (LO_T, HI_T)
    LO_T, HI_T = best
    NT = LO_T + HI_T

    per_core = []
    for c in range(NC):
        m = e_core == c
        rows_e, blk_e, col_e, cls_e = (
            e_src_row[m], e_blk[m], e_col[m], cls[m])
        order = np.lexsort((col_e, cls_e, blk_e))
        rows_e, blk_e, col_e, cls_e = (
            rows_e[order], blk_e[order], col_e[order], cls_e[order])
        bstart = np.searchsorted(blk_e, np.arange(NBLK))
        bend = np.searchsorted(blk_e, np.arange(NBLK), side="right")

        nodes = np.arange(cstart[c], cend[c])
        # dis / inv-dis in pi layout (pads -> 1 / 0)
        dis_nm = np.ones((128, NTB), np.float32)
        inv_nm = np.zeros((128, NTB), np.float32)
        dis_nm[pos[nodes], blk[nodes]] = dis[nodes]
        inv_nm[pos[nodes], blk[nodes]] = 1.0 / dis[nodes]

        lo_idx = np.zeros(NBLK * LO_T * 128, np.int64)
        hi_idx = np.zeros(NBLK * HI_T * 128, np.int64)  # already HI_BASE-offset
        s_all = np.zeros((128, NBLK * NT * 128), np.float32)
        for b in range(NBLK):
            sl = slice(bstart[b], bend[b])
            r_b, c_b, k_b = rows_e[sl], col_e[sl], cls_e[sl]
            n = len(r_b)
            n0 = int((k_b == 0).sum())
            nfb = int((k_b == 1).sum())
            take = min(LO_T * 128 - n0, nfb)
            assert take >= 0, f"block lo overflow {n0} > {LO_T*128}"
            nlo = n0 + take
            nhi = n - nlo
            assert nhi <= HI_T * 128
            for stream, cnt, off, idxarr, base_t, ibase in (
                (0, nlo, 0, lo_idx, 0, 0),
                (1, nhi, nlo, hi_idx, LO_T, HI_BASE),
            ):
                if cnt == 0:
                    continue
                rr = r_b[off:off + cnt] - ibase
                cc = c_b[off:off + cnt]
                T = LO_T if stream == 0 else HI_T
                idxarr[b * T * 128: b * T * 128 + cnt] = rr
                k = np.arange(cnt)
                p = k % 128
                t = base_t + k // 128
                scol = (b * NT + t) * 128
                real = cc >= 0
                s_all[p[real], scol[real] + cc[real]] = 1.0
                for j in np.flatnonzero(~real):  # bias slots (dense column)
                    # table bias row holds 16*conv_b (fp8 normal range);
                    # S carries the 1/16 to keep values out of subnormals
                    s_all[p[j], scol[j]:scol[j] + 128] = inv_nm[:, b] / 16.0

        x_t = np.zeros((D, NPAD), np.float32)
        x_t[:, slot[nodes]] = np.asarray(x[nodes], np.float32).T

        lens = pcounts[c * PPC:(c + 1) * PPC]
        starts = np.concatenate([[0], np.cumsum(lens)])[:-1]
        q = np.arange(ncore[c])
        pj = np.searchsorted(starts, q, side="right") - 1
        pone = np.zeros((128, NTB * PPC), bf16)
        pone[pos[nodes], blk[nodes] * PPC + pj] = 1.0
        lys_nm = np.zeros((128, NTB), np.float32)
        lys_nm[pos[nodes], blk[nodes]] = np.asarray(
            lysine_mask[nodes], np.float32)

        per_core.append(dict(
            x_t=x_t.astype(bf16),
            s_all=s_all.astype(E4),
            idx_lo=_pack_idx(lo_idx, NBLK * LO_T * 128),
            idx_hi=_pack_idx(hi_idx, NBLK * HI_T * 128),
            dis_nm=dis_nm,
            pone=pone,
            cnt_col=lens.astype(np.float32).reshape(PPC, 1),
            lys_nm=lys_nm,
        ))
    return per_core, LO_T, HI_T, NT


# ---------------------------------------------------------------- program

def _dma_gather_128(nc, out_ap, in_ap, idxs_ap, num_idxs):
    """dma_gather with a 128B payload on a 256B-stride table (elem_size=128
    fp8, elem_step=256). Bypasses bass's %256 payload assert; verified
    byte-exact on hardware."""
    g = nc.gpsimd
    _in_ap = g.lower_ap_dma(in_ap, for_custom_bir_dma=True)
    _idxs_ap = g.lower_ap(idxs_ap)
    _out_ap = g.lower_ap(out_ap)
    return g.add_instruction(mybir.InstDMAGatherAnt(
        name=g.bass.get_next_instruction_name(),
        ins=[*_in_ap, _idxs_ap, g.lower_val_access(g.to_reg(num_idxs))],
        outs=[_out_ap],
        transpose=False, num_idxs=num_idxs, elem_size=128,
        stride_bytes_256=1, gen_mode=0, single_packet=False,
        queue_num=0, sbuf_tokens_per_rank=0, sbuf_free_dim_per_rank=0,
        sbuf_free_dim_pad_per_rank=0, sbuf_byte_offset=0))


def _build_program(LO_T, HI_T, NT):
    tile_utils.max_sbuf_usage = 204 * 1024
    nc = bacc.Bacc("TRN2", target_bir_lowering=False, num_devices=NC,
                   num_swdge_queues=2)

    din = {}
    for name, shape, dt in [
        ("x_t", [D, NPAD], bft),
        ("s_all", [128, NBLK * NT * 128], fp8),
        ("idx_lo", [128, NBLK * LO_T * 8], i16),
        ("idx_hi", [128, NBLK * HI_T * 8], i16),
        ("dis_nm", [128, NTB], f32),
        ("pone", [128, NTB * PPC], bft),
        ("cnt_col", [PPC, 1], f32),
        ("lys_nm", [128, NTB], f32),
        ("convw", [D, L * D], bft),
        ("convb_pre", [128, L * D], fp8),
        ("attw_row", [1, D], f32),
        ("outw", [D, 64], f32),
        ("outb", [64, 1], f32),
    ]:
        din[name] = nc.dram_tensor(name, shape, dt, kind="ExternalInput")
    out_t = nc.dram_tensor("out_t", [64, PPC], f32, kind="ExternalOutput")
    out_h = None
    if DBG_DUMP_H:
        out_h = nc.dram_tensor("out_h", [128, NPAD], bft,
                               kind="ExternalOutput")
    out_tb = None
    if DBG_DUMP_TB:
        out_tb = nc.dram_tensor("out_tb", [128, NTB * 128], fp8,
                                kind="ExternalOutput")

    LO_SLOTS = NBLK * LO_T * 128
    HI_SLOTS = NBLK * HI_T * 128

    with tile.TileContext(nc) as tc:
        with (
            tc.tile_pool(name="glob", bufs=1) as gp,
            tc.tile_pool(name="dram", bufs=1, space="DRAM") as dram,
            tc.tile_pool(name="msgs", bufs=3) as mp,
            tc.tile_pool(name="r2", bufs=2) as rp2,
            tc.tile_pool(name="ps_w", bufs=2, space="PSUM") as ps_w,
            tc.tile_pool(name="ps_agg", bufs=3, space="PSUM") as ps_agg,
            tc.tile_pool(name="ps_tr", bufs=1, space="PSUM") as ps_tr,
            tc.tile_pool(name="ps_r", bufs=1, space="PSUM") as ps_r,
            tc.tile_pool(name="ps_p", bufs=1, space="PSUM") as ps_p,
        ):
            # resident SBUF state
            h_fm = gp.tile([D, NPAD], bft, name="h_fm")
            nc.sync.dma_start(h_fm[:], din["x_t"][:])
            h_nm = gp.tile([128, NTB, 128], bft, name="h_nm")
            staging = gp.tile([128, NTB, 128], fp8, name="staging")
            s_sb = gp.tile([128, NBLK * NT * 128], fp8, name="s_sb")
            nc.sync.dma_start(s_sb[:], din["s_all"][:])
            dis_nm = gp.tile([128, NTB], f32)
            nc.sync.dma_start(dis_nm[:], din["dis_nm"][:])
            idx_lo = gp.tile([128, LO_SLOTS // 16], i16)
            nc.sync.dma_start(idx_lo[:], din["idx_lo"][:])
            idx_hi = gp.tile([128, HI_SLOTS // 16], i16)
            nc.sync.dma_start(idx_hi[:], din["idx_hi"][:])
            convw = gp.tile([D, L * D], bft)
            nc.sync.dma_start(convw[:], din["convw"][:])
            convb_pre = gp.tile([128, L * D], fp8)
            nc.sync.dma_start(convb_pre[:], din["convb_pre"][:])
            pone = gp.tile([128, NTB * PPC], bft)
            nc.sync.dma_start(pone[:], din["pone"][:])
            lys_nm = gp.tile([128, NTB], f32)
            nc.sync.dma_start(lys_nm[:], din["lys_nm"][:])
            cnt = gp.tile([PPC, 1], f32)
            nc.sync.dma_start(cnt[:], din["cnt_col"][:])
            attw = gp.tile([1, D], f32)
            nc.sync.dma_start(attw[:], din["attw_row"][:])
            outw = gp.tile([D, 64], f32)
            nc.sync.dma_start(outw[:], din["outw"][:])
            outb = gp.tile([64, 1], f32)
            nc.sync.dma_start(outb[:], din["outb"][:])

            stripe = dram.tile([NPAD, 256], fp8)
            hws_full = dram.tile([NPADG, 256], fp8)
            tident = gp.tile([128, 128], bft)
            make_identity(nc, tident[:])
            ident = gp.tile([128, 128], f32)
            make_identity(nc, ident[:])
            ones_r = gp.tile([1, 128], f32)
            nc.vector.memset(ones_r[:], 1.0)
            ones_bf = gp.tile([128, 1], bft)
            nc.vector.memset(ones_bf[:], 1.0)

            # att_w broadcast to all partitions (ones outer product)
            psat = ps_r.tile([128, D], f32, tag="tr")
            nc.tensor.matmul(out=psat[:], lhsT=ones_r[:],
                             rhs=attw[:], start=True, stop=True)
            attrep = gp.tile([128, D], bft)
            nc.vector.tensor_copy(attrep[:], psat[:])
            sc_nm = gp.tile([128, NTB], f32)

            def emit_wcast(layer, b):
                # table chunk: staging[:, b, :] = fp8(dis * (h @ W)),
                # node-major via out = h_fm_chunk^T @ W
                pw = ps_w.tile([128, D], f32, tag="wmm")
                nc.tensor.matmul(
                    out=pw[:],
                    lhsT=h_fm[:, b * 128:(b + 1) * 128],
                    rhs=convw[:, layer * D:(layer + 1) * D],
                    start=True, stop=True)
                nc.scalar.activation(
                    staging[:, b, :], pw[:], AF.Copy,
                    scale=dis_nm[:, b:b + 1])

            def emit_bias(layer):
                # bias table row (16*conv_b) at the reserved slot (0, 49)
                nc.vector.tensor_copy(
                    staging[0:1, NBLK - 1, :],
                    convb_pre[0:1, layer * D:(layer + 1) * D])

            for b in range(NTB):
                emit_wcast(0, b)
            emit_bias(0)

            pall_mean = None
            for layer in range(DBG_LAYERS):
                last = layer == DBG_LAYERS - 1
                spm = stripe[:, 0:128].rearrange("(p k) f -> p k f", k=NTB)
                nc.sync.dma_start(spm, staging[:])
                if DBG_NO_COLL:
                    nc.gpsimd.dma_start(hws_full[0:NPAD, :], stripe[:])
                else:
                    nc.gpsimd.collective_compute(
                        "AllGather", mybir.AluOpType.bypass,
                        replica_groups=[list(range(NC))],
                        ins=[stripe.opt()], outs=[hws_full.opt()])

                # gathers issued lazily in consumption order; aggregate
                # via DoubleRow fp8 matmuls; relu epilogue with exact
                # dis[dst] as the ACT per-partition scale.
                lo_chunks, hi_chunks = {}, {}

                def get_chunk(done, ci, slots, idx, base_hi, tg):
                    if ci not in done:
                        s0 = ci * GCH
                        n = min(GCH, slots - s0)
                        m = mp.tile([128, GCH // 128, 128], fp8, tag=tg)
                        if DBG_NO_GATHER:
                            nc.vector.memset(m[:], 0.0)
                        else:
                            src_ap = (hws_full[HI_BASE:, 0:128] if base_hi
                                      else hws_full[:, 0:128])
                            _dma_gather_128(
                                nc, m[:, : n // 128, :], src_ap,
                                idx[:, s0 // 16:(s0 + n) // 16], n)
                        done[ci] = m
                    return done[ci]

                if last and not DBG_NO_READOUT:
                    # reuse the idle "wmm" ring (no W matmuls in last layer)
                    pall_mean = ps_w.tile([128, D], f32, tag="wmm")
                for b in range(NBLK):
                    acc = ps_agg.tile([128, D], f32, tag="agg")
                    for j in range(NT // 2):
                        if 2 * j < LO_T:
                            col = b * LO_T + 2 * j
                            mm = get_chunk(
                                lo_chunks, col // (GCH // 128), LO_SLOTS,
                                idx_lo, False, "mlo")
                        else:
                            col = b * HI_T + 2 * (j - LO_T // 2)
                            mm = get_chunk(
                                hi_chunks, col // (GCH // 128), HI_SLOTS,
                                idx_hi, True, "mhi")
                        cc = col % (GCH // 128)
                        sc0 = (b * NT + 2 * j) * 128
                        nc.tensor.matmul(
                            out=acc[:],
                            lhsT=s_sb[:, sc0:sc0 + 256].rearrange(
                                "p (i d) -> p i d", i=2),
                            rhs=mm[:, cc:cc + 2, :],
                            start=(j == 0), stop=(j == NT // 2 - 1),
                            perf_mode=mybir.MatmulPerfMode.DoubleRow)
                    nc.scalar.activation(
                        h_nm[:, b, :], acc[:], AF.Relu,
                        scale=dis_nm[:, b:b + 1])
                    if not last or DBG_DUMP_H:
                        pt = ps_tr.tile([128, 128], bft, tag="ptr")
                        nc.tensor.transpose(
                            out=pt[:], in_=h_nm[:, b, :],
                            identity=tident[:])
                        nc.vector.tensor_copy(
                            h_fm[:, b * 128:(b + 1) * 128], pt[:])
                    if not last:
                        # next layer's table chunk, pipelined under this
                        # layer's gather phase
                        emit_wcast(layer + 1, b)
                    elif not DBG_NO_READOUT:
                        # readout pieces that only need h_nm[b]: scores
                        # (DVE mul+reduce) and the mean-pool matmul
                        tmp = rp2.tile([128, D], bft, tag="sc")
                        nc.vector.tensor_mul(tmp[:], h_nm[:, b, :],
                                             attrep[:])
                        nc.vector.tensor_reduce(
                            out=sc_nm[:, b:b + 1], in_=tmp[:],
                            axis=mybir.AxisListType.X,
                            op=mybir.AluOpType.add)
                        nc.tensor.matmul(
                            out=pall_mean[0:PPC, :],
                            lhsT=pone[:, b * PPC:(b + 1) * PPC],
                            rhs=h_nm[:, b, :],
                            start=(b == 0), stop=(b == NBLK - 1))
                if not last:
                    emit_bias(layer + 1)

            if DBG_DUMP_H:
                for b in range(NTB):
                    nc.gpsimd.dma_start(
                        out_h[:, b * 128:(b + 1) * 128],
                        h_fm[:, b * 128:(b + 1) * 128])
            if DBG_DUMP_TB:
                nc.gpsimd.dma_start(
                    out_tb[:].rearrange("p (k f) -> p k f", k=NTB),
                    staging[:])

            if DBG_NO_READOUT:
                oz = rp2.tile([64, PPC], f32, tag="oz")
                nc.vector.tensor_copy(oz[:], h_nm[0:64, 0, 0:PPC])
                nc.gpsimd.dma_start(out_t[:], oz[:])

            if not DBG_NO_READOUT:
                # global-shift masked softmax pieces (shift-invariant)
                colmax = gp.tile([128, 1], f32)
                nc.vector.tensor_reduce(
                    out=colmax[:], in_=sc_nm[:],
                    axis=mybir.AxisListType.X, op=mybir.AluOpType.max)
                ptm = ps_r.tile([128, 128], f32, tag="tr")
                nc.tensor.transpose(
                    out=ptm[0:1, :], in_=colmax[:], identity=ident[:])
                rowmax = gp.tile([1, 128], f32)
                nc.vector.tensor_copy(rowmax[:], ptm[0:1, :])
                gmax = gp.tile([1, 1], f32)
                nc.vector.tensor_reduce(
                    out=gmax[:], in_=rowmax[:],
                    axis=mybir.AxisListType.X, op=mybir.AluOpType.max)
                ngmax = gp.tile([1, 1], f32)
                nc.vector.tensor_scalar_mul(ngmax[:], gmax[:], -1.0)
                psng = ps_r.tile([128, 128], f32, tag="tr")
                nc.tensor.matmul(out=psng[:, 0:1], lhsT=ones_r[:],
                                 rhs=ngmax[:], start=True, stop=True)
                ngcol = gp.tile([128, 1], f32)
                nc.vector.tensor_copy(ngcol[:], psng[:, 0:1])
                exm = gp.tile([128, NTB], f32)
                nc.scalar.activation(exm[:], sc_nm[:], AF.Exp,
                                     bias=ngcol[:])
                nc.vector.tensor_mul(exm[:], exm[:], lys_nm[:])

                # attention pooling: fold ex into the one-hot (pex) so
                # rhs is h_nm directly; denominator via a ones column
                pall_att = ps_w.tile([128, D], f32, tag="wmm")
                pall_den = ps_p.tile([128, 1], f32, tag="pden")
                for t in range(NTB):
                    pex = rp2.tile([128, PPC], bft, tag="pex")
                    nc.vector.tensor_scalar_mul(
                        pex[:], pone[:, t * PPC:(t + 1) * PPC],
                        exm[:, t:t + 1])
                    nc.tensor.matmul(
                        out=pall_att[0:PPC, :], lhsT=pex[:],
                        rhs=h_nm[:, t, :],
                        start=(t == 0), stop=(t == NTB - 1),
                        skip_group_check=True)
                    nc.tensor.matmul(
                        out=pall_den[0:PPC, :], lhsT=pex[:], rhs=ones_bf[:],
                        start=(t == 0), stop=(t == NTB - 1),
                        skip_group_check=True)

                # c_j = 1/(max(cnt,1)*sqrt(cnt+1e-6)); rden = 1/max(dn,eps)
                cg = gp.tile([PPC, 1], f32)
                nc.vector.tensor_scalar_max(cg[:], cnt[:], 1.0)
                cnte = gp.tile([PPC, 1], f32)
                nc.vector.tensor_scalar_add(cnte[:], cnt[:], 1.0e-6)
                sq = gp.tile([PPC, 1], f32)
                nc.scalar.activation(sq[:], cnte[:], AF.Sqrt)
                mm_ = gp.tile([PPC, 1], f32)
                nc.vector.tensor_mul(mm_[:], cg[:], sq[:])
                cj = gp.tile([PPC, 1], f32)
                nc.vector.reciprocal(cj[:], mm_[:])
                dg = gp.tile([PPC, 1], f32)
                nc.vector.tensor_scalar_max(dg[:], pall_den[0:PPC, :],
                                            1.0e-30)
                rden = gp.tile([PPC, 1], f32)
                nc.vector.reciprocal(rden[:], dg[:])

                pre = gp.tile([PPC, 128], f32)
                nc.vector.tensor_scalar_mul(pre[:], pall_mean[0:PPC, :],
                                            cj[:])
                lw = gp.tile([PPC, 128], f32)
                nc.vector.tensor_scalar_mul(lw[:], pall_att[0:PPC, :],
                                            rden[:])
                nc.vector.tensor_add(pre[:], pre[:], lw[:])

                # out^T = outw^T @ pre^T + outb
                ptp = ps_r.tile([128, 128], f32, tag="tr")
                nc.tensor.transpose(
                    out=ptp[:, 0:PPC], in_=pre[:],
                    identity=ident[0:PPC, 0:PPC])
                preT = gp.tile([128, PPC], f32)
                nc.vector.tensor_copy(preT[:], ptp[:, 0:PPC])
                pso = ps_r.tile([128, 128], f32, tag="tr")
                nc.tensor.matmul(
                    out=pso[0:64, 0:PPC], lhsT=outw[:], rhs=preT[:],
                    start=True, stop=True)
                osb = gp.tile([64, PPC], f32)
                nc.vector.tensor_scalar_add(osb[:], pso[0:64, 0:PPC],
                                            outb[:])
                nc.gpsimd.dma_start(out_t[:], osb[:])

    nc.compile()
    return nc


# ---------------------------------------------------------------- entry

def kernel(**inputs):
    x = np.asarray(inputs["x"], np.float32)
    edge_index = np.asarray(inputs["edge_index"])
    batch = np.asarray(inputs["batch"])
    lysine_mask = np.asarray(inputs["lysine_mask"])
    conv_w = np.asarray(inputs["conv_w"], np.float32)
    conv_b = np.asarray(inputs["conv_b"], np.float32)
    att_w = np.asarray(inputs["att_w"], np.float32)
    out_w = np.asarray(inputs["out_w"], np.float32)
    out_b = np.asarray(inputs["out_b"], np.float32)

    per_core, LO_T, HI_T, NT = _host_prep(x, edge_index, batch, lysine_mask)

    convw = np.ascontiguousarray(
        np.concatenate([conv_w[i] for i in range(L)], axis=1)).astype(bf16)
    convb_pre = np.tile(
        np.concatenate([16.0 * conv_b[i] for i in range(L)]).astype(E4),
        (128, 1))
    shared = dict(
        convw=convw, convb_pre=convb_pre,
        attw_row=att_w.reshape(1, D).astype(np.float32),
        outw=out_w.astype(np.float32),
        outb=out_b.reshape(64, 1).astype(np.float32),
    )
    in_maps = []
    for c in range(NC):
        pc = per_core[c]
        in_maps.append({
            "x_t": pc["x_t"], "s_all": pc["s_all"],
            "idx_lo": pc["idx_lo"], "idx_hi": pc["idx_hi"],
            "dis_nm": pc["dis_nm"],
            "pone": pc["pone"], "cnt_col": pc["cnt_col"],
            "lys_nm": pc["lys_nm"], **shared,
        })

    nc_prog = _build_program(LO_T, HI_T, NT)
    trace = os.environ.get("GCN_TRACE", "") == "1"
    res = run_bass_kernel_spmd(
        nc_prog, in_maps, core_ids=list(range(NC)), trace=trace)
    if trace:
        import kernel as _self
        _self.LAST_RESULT = res
        print("HW exec time:", res.exec_time_ns, "ns")
    out = np.concatenate(
        [np.asarray(res.results[c]["out_t"], np.float32).T for c in range(NC)],
        axis=0)
    return out


# revision 19
# speedup vs baseline: 1.2776x; 1.0837x over previous
"""GCN message-passing kernel for Trainium2, 8 NeuronCores (SPMD).

Strategy (graph-parallel, fp8 messages):
- Nodes are protein-contiguous, sharded across 8 cores at protein boundaries
  (16 proteins/core, padded to 6400 nodes/core). Within a core, nodes are
  bin-packed into 50 blocks of 128 balancing incoming-edge counts; slot
  s = blk*128 + pos maps to partition pos, chunk blk everywhere (h, dis,
  table, S, pooling) so aggregation blocks coincide with node chunks.
- Message table is fp8e4m3 at 256B row stride with a 128B payload
  (row = dis[src] * (h @ W)[src]); gathers use elem_size=128/elem_step=256
  (bass's %256 payload assert is bypassed via direct InstDMAGatherAnt
  construction - verified byte-exact on hardware), halving per-edge DMA
  cost vs bf16.
- Aggregation is node-major: acc[dst,feat] = S^T @ msgs with S a 0/1
  one-hot (exact in fp8) via DoubleRow fp8 matmuls (256 slots each);
  dis[dst] is applied EXACTLY by the relu epilogue's per-partition ACT
  scale; conv bias enters through one bias slot per block whose S column
  is 1/dis[dst] and whose table row is conv_b (rewritten per layer at the
  reserved slot (pos 0, blk 49) of every core).
- h is kept bf16 both node-major (epilogue output; feeds readout) and
  feature-major (one PE transpose per chunk per layer; feeds the bf16
  h @ W matmuls computed directly node-major as lhsT=h_fm, rhs=W).
- Readout: scores via DVE mul+reduce against a broadcast att_w row,
  global-shift masked softmax, fused pooling matmuls with
  rhs = [h | ex*h | ex] per chunk, final projection per core.
"""
import os
import numpy as np
import ml_dtypes

DBG_LAYERS = int(os.environ.get("GCN_DBG_LAYERS", "4"))
DBG_NO_COLL = os.environ.get("GCN_DBG_NO_COLL", "") == "1"
DBG_NO_GATHER = os.environ.get("GCN_DBG_NO_GATHER", "") == "1"
DBG_NO_READOUT = os.environ.get("GCN_DBG_NO_READOUT", "") == "1"
DBG_DUMP_H = os.environ.get("GCN_DBG_DUMP_H", "") == "1"
DBG_DUMP_TB = os.environ.get("GCN_DBG_DUMP_TB", "") == "1"

import concourse.bacc as bacc
import concourse.tile as tile
import concourse.tile_utils as tile_utils
from concourse import mybir
from concourse.bass_utils import run_bass_kernel_spmd
from concourse.masks import make_identity

bf16 = ml_dtypes.bfloat16
E4 = ml_dtypes.float8_e4m3
AF = mybir.ActivationFunctionType

NC = 8
D = 128
L = 4
B = 128
PPC = B // NC          # proteins per core
NPAD = 6400            # padded nodes per core
NPADG = NC * NPAD      # global padded rows
NTB = NPAD // 128      # 50 chunks of 128 nodes == aggregation blocks
NBLK = NTB
TW = 128               # dst nodes per aggregation block
LO_BOUND = 32000       # lo gather covers rows [0, 32000)
HI_BASE = 18560        # hi gather covers rows [18560, 51200): 32639 <= int16
GCH = 8192             # gather slots per dma_gather instruction (64 cols)
BIAS_SLOT = NBLK - 1   # reserved slot (pos 0, blk 49) on every core

f32 = mybir.dt.float32
bft = mybir.dt.bfloat16
fp8 = mybir.dt.float8e4
i16 = mybir.dt.int16


# ---------------------------------------------------------------- host prep

def _pack_idx(vals, slots):
    """int16 gather index layout: position i -> partition i%16, col i//16,
    replicated across the 128 partitions."""
    assert len(vals) == slots and slots % 16 == 0
    arr = np.asarray(vals, np.int16).reshape(slots // 16, 16).T  # [16, s//16]
    return np.ascontiguousarray(np.tile(arr, (8, 1)))


def _ceil128(x):
    return max(1, int(np.ceil(x / 128)))


def _host_prep(x, edge_index, batch, lysine_mask):
    N = x.shape[0]
    src = np.asarray(edge_index[0], np.int64)
    dst = np.asarray(edge_index[1], np.int64)
    batch = np.asarray(batch, np.int64)

    pcounts = np.bincount(batch, minlength=B)
    pstart = np.concatenate([[0], np.cumsum(pcounts)])
    cstart = pstart[np.arange(NC) * PPC]
    cend = pstart[(np.arange(NC) + 1) * PPC]
    ncore = cend - cstart
    assert ncore.max() <= NPAD - 1, f"core node count {ncore.max()} > {NPAD-1}"
    assert pcounts.max() <= 128 * NTB

    deg = np.bincount(dst, minlength=N).astype(np.float64) + 1.0
    dis = (1.0 / np.sqrt(deg)).astype(np.float32)
    core_of = np.searchsorted(cend, np.arange(N), side="right")

    # --- per-core node packing into NBLK blocks of 128, balancing in-slot
    # (in-edges + self) counts per block; (pos 127, blk 49) is reserved.
    blk = np.zeros(N, np.int64)
    pos = np.zeros(N, np.int64)
    for c in range(NC):
        nodes = np.arange(cstart[c], cend[c])
        tot = deg[nodes]
        order = np.argsort(-tot, kind="stable")
        caps = np.full(NBLK, 128, np.int64)
        caps[NBLK - 1] = 127
        loads = np.zeros(NBLK)
        cnts = np.zeros(NBLK, np.int64)
        for i in order:
            masked = np.where(cnts < caps, loads, np.inf)
            b = int(np.argmin(masked))
            blk[nodes[i]] = b
            # (pos 0, blk 49) is the reserved bias slot on every core
            pos[nodes[i]] = cnts[b] + (1 if b == NBLK - 1 else 0)
            cnts[b] += 1
            loads[b] += tot[i]
    slot = blk * 128 + pos                    # local pi slot
    grow = core_of * NPAD + pos * NTB + blk   # global table row

    # --- edge list: real edges + self edges + one bias pseudo-edge per
    # (core, block) (dst col -1). Bias row is the reserved slot's row.
    e_src_row = np.concatenate([grow[src], grow])
    e_dst = np.concatenate([dst, np.arange(N)])
    e_core = core_of[e_dst]
    e_blk = blk[e_dst]
    e_col = pos[e_dst]
    bias_core = np.repeat(np.arange(NC), NBLK)
    bias_blk = np.tile(np.arange(NBLK), NC)
    bias_row_of_core = np.arange(NC) * NPAD + (NBLK - 1)  # (pos 0, blk 49)
    e_src_row = np.concatenate([e_src_row, bias_row_of_core[bias_core]])
    e_core = np.concatenate([e_core, bias_core])
    e_blk = np.concatenate([e_blk, bias_blk])
    e_col = np.concatenate([e_col, np.full(NC * NBLK, -1, np.int64)])

    cls = np.where(e_src_row < HI_BASE, 0,
                   np.where(e_src_row < LO_BOUND, 1, 2))
    key = e_core * NBLK + e_blk
    nl0 = np.bincount(key[cls == 0], minlength=NC * NBLK)
    nf = np.bincount(key[cls == 1], minlength=NC * NBLK)
    tot_cb = np.bincount(key, minlength=NC * NBLK)

    best = None
    for LO_T in range(_ceil128(nl0.max()), _ceil128(nl0.max()) + 4):
        lo_fill = np.minimum(LO_T * 128, nl0 + nf)
        HI_T = _ceil128((tot_cb - lo_fill).max())
        if best is None or LO_T + HI_T < best[0] + best[1]:
            best = (LO_T, HI_T)
    LO_T, HI_T = best
    NT = LO_T + HI_T

    per_core = []
    for c in range(NC):
        m = e_core == c
        rows_e, blk_e, col_e, cls_e = (
            e_src_row[m], e_blk[m], e_col[m], cls[m])
        order = np.lexsort((col_e, cls_e, blk_e))
        rows_e, blk_e, col_e, cls_e = (
            rows_e[order], blk_e[order], col_e[order], cls_e[order])
        bstart = np.searchsorted(blk_e, np.arange(NBLK))
        bend = np.searchsorted(blk_e, np.arange(NBLK), side="right")

        nodes = np.arange(cstart[c], cend[c])
        # dis / inv-dis in pi layout (pads -> 1 / 0)
        dis_nm = np.ones((128, NTB), np.float32)
        inv_nm = np.zeros((128, NTB), np.float32)
        dis_nm[pos[nodes], blk[nodes]] = dis[nodes]
        inv_nm[pos[nodes], blk[nodes]] = 1.0 / dis[nodes]

        lo_idx = np.zeros(NBLK * LO_T * 128, np.int64)
        hi_idx = np.zeros(NBLK * HI_T * 128, np.int64)  # already HI_BASE-offset
        s_all = np.zeros((128, NBLK * NT * 128), np.float32)
        for b in range(NBLK):
            sl = slice(bstart[b], bend[b])
            r_b, c_b, k_b = rows_e[sl], col_e[sl], cls_e[sl]
            n = len(r_b)
            n0 = int((k_b == 0).sum())
            nfb = int((k_b == 1).sum())
            take = min(LO_T * 128 - n0, nfb)
            assert take >= 0, f"block lo overflow {n0} > {LO_T*128}"
            nlo = n0 + take
            nhi = n - nlo
            assert nhi <= HI_T * 128
            for stream, cnt, off, idxarr, base_t, ibase in (
                (0, nlo, 0, lo_idx, 0, 0),
                (1, nhi, nlo, hi_idx, LO_T, HI_BASE),
            ):
                if cnt == 0:
                    continue
                rr = r_b[off:off + cnt] - ibase
                cc = c_b[off:off + cnt]
                T = LO_T if stream == 0 else HI_T
                idxarr[b * T * 128: b * T * 128 + cnt] = rr
                k = np.arange(cnt)
                p = k % 128
                t = base_t + k // 128
                scol = (b * NT + t) * 128
                real = cc >= 0
                s_all[p[real], scol[real] + cc[real]] = 1.0
                for j in np.flatnonzero(~real):  # bias slots (dense column)
                    # table bias row holds 16*conv_b (fp8 normal range);
                    # S carries the 1/16 to keep values out of subnormals
                    s_all[p[j], scol[j]:scol[j] + 128] = inv_nm[:, b] / 16.0

        x_t = np.zeros((D, NPAD), np.float32)
        x_t[:, slot[nodes]] = np.asarray(x[nodes], np.float32).T

        lens = pcounts[c * PPC:(c + 1) * PPC]
        starts = np.concatenate([[0], np.cumsum(lens)])[:-1]
        q = np.arange(ncore[c])
        pj = np.searchsorted(starts, q, side="right") - 1
        pone = np.zeros((128, NTB * PPC), bf16)
        pone[pos[nodes], blk[nodes] * PPC + pj] = 1.0
        lys_nm = np.zeros((128, NTB), np.float32)
        lys_nm[pos[nodes], blk[nodes]] = np.asarray(
            lysine_mask[nodes], np.float32)

        per_core.append(dict(
            x_t=x_t.astype(bf16),
            s_all=s_all.astype(E4),
            idx_lo=_pack_idx(lo_idx, NBLK * LO_T * 128),
            idx_hi=_pack_idx(hi_idx, NBLK * HI_T * 128),
            dis_nm=dis_nm,
            pone=pone,
            cnt_col=lens.astype(np.float32).reshape(PPC, 1),
            lys_nm=lys_nm,
        ))
    return per_core, LO_T, HI_T, NT


# ---------------------------------------------------------------- program

def _dma_gather_128(nc, out_ap, in_ap, idxs_ap, num_idxs):
    """dma_gather with a 128B payload on a 256B-stride table (elem_size=128
    fp8, elem_step=256). Bypasses bass's %256 payload assert; verified
    byte-exact on hardware."""
    g = nc.gpsimd
    _in_ap = g.lower_ap_dma(in_ap, for_custom_bir_dma=True)
    _idxs_ap = g.lower_ap(idxs_ap)
    _out_ap = g.lower_ap(out_ap)
    return g.add_instruction(mybir.InstDMAGatherAnt(
        name=g.bass.get_next_instruction_name(),
        ins=[*_in_ap, _idxs_ap, g.lower_val_access(g.to_reg(num_idxs))],
        outs=[_out_ap],
        transpose=False, num_idxs=num_idxs, elem_size=128,
        stride_bytes_256=1, gen_mode=0, single_packet=False,
        queue_num=0, sbuf_tokens_per_rank=0, sbuf_free_dim_per_rank=0,
        sbuf_free_dim_pad_per_rank=0, sbuf_byte_offset=0))


def _build_program(LO_T, HI_T, NT):
    tile_utils.max_sbuf_usage = 204 * 1024
    nc = bacc.Bacc("TRN2", target_bir_lowering=False, num_devices=NC,
                   num_swdge_queues=2)

    din = {}
    for name, shape, dt in [
        ("x_t", [D, NPAD], bft),
        ("s_all", [128, NBLK * NT * 128], fp8),
        ("idx_lo", [128, NBLK * LO_T * 8], i16),
        ("idx_hi", [128, NBLK * HI_T * 8], i16),
        ("dis_nm", [128, NTB], f32),
        ("pone", [128, NTB * PPC], bft),
        ("cnt_col", [PPC, 1], f32),
        ("lys_nm", [128, NTB], f32),
        ("convw", [D, L * D], bft),
        ("convb_pre", [128, L * D], fp8),
        ("attw_row", [1, D], f32),
        ("outw", [D, 64], f32),
        ("outb", [64, 1], f32),
    ]:
        din[name] = nc.dram_tensor(name, shape, dt, kind="ExternalInput")
    out_t = nc.dram_tensor("out_t", [64, PPC], f32, kind="ExternalOutput")
    out_h = None
    if DBG_DUMP_H:
        out_h = nc.dram_tensor("out_h", [128, NPAD], bft,
                               kind="ExternalOutput")
    out_tb = None
    if DBG_DUMP_TB:
        out_tb = nc.dram_tensor("out_tb", [128, NTB * 128], fp8,
                                kind="ExternalOutput")

    LO_SLOTS = NBLK * LO_T * 128
    HI_SLOTS = NBLK * HI_T * 128

    with tile.TileContext(nc) as tc:
        with (
            tc.tile_pool(name="glob", bufs=1) as gp,
            tc.tile_pool(name="dram", bufs=1, space="DRAM") as dram,
            tc.tile_pool(name="msgs", bufs=3) as mp,
            tc.tile_pool(name="r2", bufs=2) as rp2,
            tc.tile_pool(name="ps_w", bufs=2, space="PSUM") as ps_w,
            tc.tile_pool(name="ps_agg", bufs=3, space="PSUM") as ps_agg,
            tc.tile_pool(name="ps_tr", bufs=1, space="PSUM") as ps_tr,
            tc.tile_pool(name="ps_r", bufs=1, space="PSUM") as ps_r,
            tc.tile_pool(name="ps_p", bufs=1, space="PSUM") as ps_p,
        ):
            # resident SBUF state
            h_fm = gp.tile([D, NPAD], bft, name="h_fm")
            nc.sync.dma_start(h_fm[:], din["x_t"][:])
            h_nm = gp.tile([128, NTB, 128], bft, name="h_nm")
            staging = gp.tile([128, NTB, 128], fp8, name="staging")
            s_sb = gp.tile([128, NBLK * NT * 128], fp8, name="s_sb")
            nc.sync.dma_start(s_sb[:], din["s_all"][:])
            dis_nm = gp.tile([128, NTB], f32)
            nc.sync.dma_start(dis_nm[:], din["dis_nm"][:])
            idx_lo = gp.tile([128, LO_SLOTS // 16], i16)
            nc.sync.dma_start(idx_lo[:], din["idx_lo"][:])
            idx_hi = gp.tile([128, HI_SLOTS // 16], i16)
            nc.sync.dma_start(idx_hi[:], din["idx_hi"][:])
            convw = gp.tile([D, L * D], bft)
            nc.sync.dma_start(convw[:], din["convw"][:])
            convb_pre = gp.tile([128, L * D], fp8)
            nc.sync.dma_start(convb_pre[:], din["convb_pre"][:])
            pone = gp.tile([128, NTB * PPC], bft)
            nc.sync.dma_start(pone[:], din["pone"][:])
            lys_nm = gp.tile([128, NTB], f32)
            nc.sync.dma_start(lys_nm[:], din["lys_nm"][:])
            cnt = gp.tile([PPC, 1], f32)
            nc.sync.dma_start(cnt[:], din["cnt_col"][:])
            attw = gp.tile([1, D], f32)
            nc.sync.dma_start(attw[:], din["attw_row"][:])
            outw = gp.tile([D, 64], f32)
            nc.sync.dma_start(outw[:], din["outw"][:])
            outb = gp.tile([64, 1], f32)
            nc.sync.dma_start(outb[:], din["outb"][:])

            stripe = dram.tile([NPAD, 256], fp8)
            hws_full = dram.tile([NPADG, 256], fp8)
            tident = gp.tile([128, 128], bft)
            make_identity(nc, tident[:])
            ident = gp.tile([128, 128], f32)
            make_identity(nc, ident[:])
            ones_r = gp.tile([1, 128], f32)
            nc.vector.memset(ones_r[:], 1.0)
            ones_bf = gp.tile([128, 1], bft)
            nc.vector.memset(ones_bf[:], 1.0)

            # att_w broadcast to all partitions (ones outer product)
            psat = ps_r.tile([128, D], f32, tag="tr")
            nc.tensor.matmul(out=psat[:], lhsT=ones_r[:],
                             rhs=attw[:], start=True, stop=True)
            attrep = gp.tile([128, D], bft)
            nc.vector.tensor_copy(attrep[:], psat[:])
            sc_nm = gp.tile([128, NTB], f32)

            def emit_wcast(layer, b):
                # table chunk: staging[:, b, :] = fp8(dis * (h @ W)),
                # node-major via out = h_fm_chunk^T @ W
                pw = ps_w.tile([128, D], f32, tag="wmm")
                nc.tensor.matmul(
                    out=pw[:],
                    lhsT=h_fm[:, b * 128:(b + 1) * 128],
                    rhs=convw[:, layer * D:(layer + 1) * D],
                    start=True, stop=True)
                nc.scalar.activation(
                    staging[:, b, :], pw[:], AF.Copy,
                    scale=dis_nm[:, b:b + 1])

            def emit_bias(layer):
                # bias table row (16*conv_b) at the reserved slot (0, 49)
                nc.vector.tensor_copy(
                    staging[0:1, NBLK - 1, :],
                    convb_pre[0:1, layer * D:(layer + 1) * D])

            for b in range(NTB):
                emit_wcast(0, b)
            emit_bias(0)

            pall_mean = None
            for layer in range(DBG_LAYERS):
                last = layer == DBG_LAYERS - 1
                spm = stripe[:, 0:128].rearrange("(p k) f -> p k f", k=NTB)
                nc.sync.dma_start(spm, staging[:])
                if DBG_NO_COLL:
                    nc.gpsimd.dma_start(hws_full[0:NPAD, :], stripe[:])
                else:
                    nc.gpsimd.collective_compute(
                        "AllGather", mybir.AluOpType.bypass,
                        replica_groups=[list(range(NC))],
                        ins=[stripe.opt()], outs=[hws_full.opt()])

                # gathers issued lazily in consumption order; aggregate
                # via DoubleRow fp8 matmuls; relu epilogue with exact
                # dis[dst] as the ACT per-partition scale.
                lo_chunks, hi_chunks = {}, {}

                def get_chunk(done, ci, slots, idx, base_hi, tg):
                    if ci not in done:
                        s0 = ci * GCH
                        n = min(GCH, slots - s0)
                        m = mp.tile([128, GCH // 128, 128], fp8, tag=tg)
                        if DBG_NO_GATHER:
                            nc.vector.memset(m[:], 0.0)
                        else:
                            src_ap = (hws_full[HI_BASE:, 0:128] if base_hi
                                      else hws_full[:, 0:128])
                            _dma_gather_128(
                                nc, m[:, : n // 128, :], src_ap,
                                idx[:, s0 // 16:(s0 + n) // 16], n)
                        done[ci] = m
                    return done[ci]

                if last and not DBG_NO_READOUT:
                    # reuse the idle "wmm" ring (no W matmuls in last layer)
                    pall_mean = ps_w.tile([128, D], f32, tag="wmm")
                CCH = GCH // 128
                for b in range(NBLK):
                    acc = ps_agg.tile([128, D], f32, tag="agg")
                    # plan matmuls: DoubleRow pairs where chunk-aligned,
                    # plain fp8 matmuls for odd tails / chunk straddles
                    ops = []
                    for T, base_t, st in ((LO_T, 0, 0), (HI_T, LO_T, 1)):
                        t = 0
                        while t < T:
                            col = b * T + t
                            if t + 1 < T and (col % CCH) != CCH - 1:
                                ops.append((st, T, base_t, t, 2))
                                t += 2
                            else:
                                ops.append((st, T, base_t, t, 1))
                                t += 1
                    for k, (st, T, base_t, t, w) in enumerate(ops):
                        col = b * T + t
                        if st == 0:
                            mm = get_chunk(lo_chunks, col // CCH, LO_SLOTS,
                                           idx_lo, False, "mlo")
                        else:
                            mm = get_chunk(hi_chunks, col // CCH, HI_SLOTS,
                                           idx_hi, True, "mhi")
                        cc = col % CCH
                        sc0 = (b * NT + base_t + t) * 128
                        if w == 2:
                            nc.tensor.matmul(
                                out=acc[:],
                                lhsT=s_sb[:, sc0:sc0 + 256].rearrange(
                                    "p (i d) -> p i d", i=2),
                                rhs=mm[:, cc:cc + 2, :],
                                start=(k == 0), stop=(k == len(ops) - 1),
                                perf_mode=mybir.MatmulPerfMode.DoubleRow)
                        else:
                            nc.tensor.matmul(
                                out=acc[:],
                                lhsT=s_sb[:, sc0:sc0 + 128],
                                rhs=mm[:, cc, :],
                                start=(k == 0), stop=(k == len(ops) - 1))
                    nc.scalar.activation(
                        h_nm[:, b, :], acc[:], AF.Relu,
                        scale=dis_nm[:, b:b + 1])
                    if not last or DBG_DUMP_H:
                        pt = ps_tr.tile([128, 128], bft, tag="ptr")
                        nc.tensor.transpose(
                            out=pt[:], in_=h_nm[:, b, :],
                            identity=tident[:])
                        nc.vector.tensor_copy(
                            h_fm[:, b * 128:(b + 1) * 128], pt[:])
                    if not last:
                        # next layer's table chunk, pipelined under this
                        # layer's gather phase
                        emit_wcast(layer + 1, b)
                    elif not DBG_NO_READOUT:
                        # readout pieces that only need h_nm[b]: scores
                        # (DVE mul+reduce) and the mean-pool matmul
                        tmp = rp2.tile([128, D], bft, tag="sc")
                        nc.vector.tensor_mul(tmp[:], h_nm[:, b, :],
                                             attrep[:])
                        nc.vector.tensor_reduce(
                            out=sc_nm[:, b:b + 1], in_=tmp[:],
                            axis=mybir.AxisListType.X,
                            op=mybir.AluOpType.add)
                        nc.tensor.matmul(
                            out=pall_mean[0:PPC, :],
                            lhsT=pone[:, b * PPC:(b + 1) * PPC],
                            rhs=h_nm[:, b, :],
                            start=(b == 0), stop=(b == NBLK - 1))
                if not last:
                    emit_bias(layer + 1)

            if DBG_DUMP_H:
                for b in range(NTB):
                    nc.gpsimd.dma_start(
                        out_h[:, b * 128:(b + 1) * 128],
                        h_fm[:, b * 128:(b + 1) * 128])
            if DBG_DUMP_TB:
                nc.gpsimd.dma_start(
                    out_tb[:].rearrange("p (k f) -> p k f", k=NTB),
                    staging[:])

            if DBG_NO_READOUT:
                oz = rp2.tile([64, PPC], f32, tag="oz")
                nc.vector.tensor_copy(oz[:], h_nm[0:64, 0, 0:PPC])
                nc.gpsimd.dma_start(out_t[:], oz[:])

            if not DBG_NO_READOUT:
                # global-shift masked softmax pieces (shift-invariant)
                colmax = gp.tile([128, 1], f32)
                nc.vector.tensor_reduce(
                    out=colmax[:], in_=sc_nm[:],
                    axis=mybir.AxisListType.X, op=mybir.AluOpType.max)
                ptm = ps_r.tile([128, 128], f32, tag="tr")
                nc.tensor.transpose(
                    out=ptm[0:1, :], in_=colmax[:], identity=ident[:])
                rowmax = gp.tile([1, 128], f32)
                nc.vector.tensor_copy(rowmax[:], ptm[0:1, :])
                gmax = gp.tile([1, 1], f32)
                nc.vector.tensor_reduce(
                    out=gmax[:], in_=rowmax[:],
                    axis=mybir.AxisListType.X, op=mybir.AluOpType.max)
                ngmax = gp.tile([1, 1], f32)
                nc.vector.tensor_scalar_mul(ngmax[:], gmax[:], -1.0)
                psng = ps_r.tile([128, 128], f32, tag="tr")
                nc.tensor.matmul(out=psng[:, 0:1], lhsT=ones_r[:],
                                 rhs=ngmax[:], start=True, stop=True)
                ngcol = gp.tile([128, 1], f32)
                nc.vector.tensor_copy(ngcol[:], psng[:, 0:1])
                exm = gp.tile([128, NTB], f32)
                nc.scalar.activation(exm[:], sc_nm[:], AF.Exp,
                                     bias=ngcol[:])
                nc.vector.tensor_mul(exm[:], exm[:], lys_nm[:])

                # attention pooling: fold ex into the one-hot (pex) so
                # rhs is h_nm directly; denominator via a ones column
                pall_att = ps_w.tile([128, D], f32, tag="wmm")
                pall_den = ps_p.tile([128, 1], f32, tag="pden")
                pex_all = gp.tile([128, NTB, PPC], bft)
                nc.vector.tensor_mul(
                    pex_all[:],
                    pone[:].rearrange("p (k o) -> p k o", o=PPC),
                    exm[:].broadcast_to([128, NTB, PPC]))
                for t in range(NTB):
                    nc.tensor.matmul(
                        out=pall_att[0:PPC, :], lhsT=pex_all[:, t, :],
                        rhs=h_nm[:, t, :],
                        start=(t == 0), stop=(t == NTB - 1),
                        skip_group_check=True)
                    nc.tensor.matmul(
                        out=pall_den[0:PPC, :], lhsT=pex_all[:, t, :],
                        rhs=ones_bf[:],
                        start=(t == 0), stop=(t == NTB - 1),
                        skip_group_check=True)

                # c_j = 1/(max(cnt,1)*sqrt(cnt+1e-6)); rden = 1/max(dn,eps)
                cg = gp.tile([PPC, 1], f32)
                nc.vector.tensor_scalar_max(cg[:], cnt[:], 1.0)
                cnte = gp.tile([PPC, 1], f32)
                nc.vector.tensor_scalar_add(cnte[:], cnt[:], 1.0e-6)
                sq = gp.tile([PPC, 1], f32)
                nc.scalar.activation(sq[:], cnte[:], AF.Sqrt)
                mm_ = gp.tile([PPC, 1], f32)
                nc.vector.tensor_mul(mm_[:], cg[:], sq[:])
                cj = gp.tile([PPC, 1], f32)
                nc.vector.reciprocal(cj[:], mm_[:])
                dg = gp.tile([PPC, 1], f32)
                nc.vector.tensor_scalar_max(dg[:], pall_den[0:PPC, :],
                                            1.0e-30)
                rden = gp.tile([PPC, 1], f32)
                nc.vector.reciprocal(rden[:], dg[:])

                pre = gp.tile([PPC, 128], f32)
                nc.vector.tensor_scalar_mul(pre[:], pall_mean[0:PPC, :],
                                            cj[:])
                lw = gp.tile([PPC, 128], f32)
                nc.vector.tensor_scalar_mul(lw[:], pall_att[0:PPC, :],
                                            rden[:])
                nc.vector.tensor_add(pre[:], pre[:], lw[:])

                # out^T = outw^T @ pre^T + outb
                ptp = ps_r.tile([128, 128], f32, tag="tr")
                nc.tensor.transpose(
                    out=ptp[:, 0:PPC], in_=pre[:],
                    identity=ident[0:PPC, 0:PPC])
                preT = gp.tile([128, PPC], f32)
                nc.vector.tensor_copy(preT[:], ptp[:, 0:PPC])
                pso = ps_r.tile([128, 128], f32, tag="tr")
                nc.tensor.matmul(
                    out=pso[0:64, 0:PPC], lhsT=outw[:], rhs=preT[:],
                    start=True, stop=True)
                osb = gp.tile([64, PPC], f32)
                nc.vector.tensor_scalar_add(osb[:], pso[0:64, 0:PPC],
                                            outb[:])
                nc.gpsimd.dma_start(out_t[:], osb[:])

    nc.compile()
    return nc


# ---------------------------------------------------------------- entry

def kernel(**inputs):
    x = np.asarray(inputs["x"], np.float32)
    edge_index = np.asarray(inputs["edge_index"])
    batch = np.asarray(inputs["batch"])
    lysine_mask = np.asarray(inputs["lysine_mask"])
    conv_w = np.asarray(inputs["conv_w"], np.float32)
    conv_b = np.asarray(inputs["conv_b"], np.float32)
    att_w = np.asarray(inputs["att_w"], np.float32)
    out_w = np.asarray(inputs["out_w"], np.float32)
    out_b = np.asarray(inputs["out_b"], np.float32)

    per_core, LO_T, HI_T, NT = _host_prep(x, edge_index, batch, lysine_mask)

    convw = np.ascontiguousarray(
        np.concatenate([conv_w[i] for i in range(L)], axis=1)).astype(bf16)
    convb_pre = np.tile(
        np.concatenate([16.0 * conv_b[i] for i in range(L)]).astype(E4),
        (128, 1))
    shared = dict(
        convw=convw, convb_pre=convb_pre,
        attw_row=att_w.reshape(1, D).astype(np.float32),
        outw=out_w.astype(np.float32),
        outb=out_b.reshape(64, 1).astype(np.float32),
    )
    in_maps = []
    for c in range(NC):
        pc = per_core[c]
        in_maps.append({
            "x_t": pc["x_t"], "s_all": pc["s_all"],
            "idx_lo": pc["idx_lo"], "idx_hi": pc["idx_hi"],
            "dis_nm": pc["dis_nm"],
            "pone": pc["pone"], "cnt_col": pc["cnt_col"],
            "lys_nm": pc["lys_nm"], **shared,
        })

    nc_prog = _build_program(LO_T, HI_T, NT)
    trace = os.environ.get("GCN_TRACE", "") == "1"
    res = run_bass_kernel_spmd(
        nc_prog, in_maps, core_ids=list(range(NC)), trace=trace)
    if trace:
        import kernel as _self
        _self.LAST_RESULT = res
        print("HW exec time:", res.exec_time_ns, "ns")
    out = np.concatenate(
        [np.asarray(res.results[c]["out_t"], np.float32).T for c in range(NC)],
        axis=0)
    return out
